# revision 7
# baseline (speedup 1.0000x reference)
"""GQA kernel for trn2, 8 NeuronCores, tensor-parallel over heads.

Sharding: 4 q heads + 1 kv head per core (column-split Wq/Wk/Wv, row-split Wo),
partial outputs summed on host. All matmul operands fp16, fp32 PSUM accumulation.

Layout strategy (everything "T" = feature-on-partitions):
  qT/kT/vT projections:  out[f, s] = W[:, f].T @ xT[:, s]  (xT host-pretransposed)
  scoresT[ks, qs] = kroT.T @ qroT  per 128-wide ks chunk    (contraction over head_dim)
  exp via ACT with per-partition scale = 0.125 * rsqrt(ssq_k) (k RMS-norm folded in)
  outT[d, qs] accumulated as [v|1].T @ expT  -> row 64 is the softmax denominator
  y[s, :] = attn_outT.T @ Wo  per 128-row seq chunk
RoPE + QK-norm gains folded into host-precomputed cos/sin tables.
"""

import sys
import types
import numpy as np

for _p in ("/opt/trn_rl_repo",):
    if _p not in sys.path:
        sys.path.append(_p)

SEQ = 2048
DIM = 2048
HD = 64
NCORES = 8
HPC = 4  # q heads per core
EPS = 1e-6
THETA = 10000.0
CB = 1024  # attention column block (2 PSUM banks wide)

_CACHE = {}


def _ensure_ntff_hook():
    """Re-register the NTFF profile hook the boot drops (stub antenv)."""
    if "antenv.axon_hooks" in sys.modules:
        return
    try:
        import antenv
        m = types.ModuleType("antenv.axon_hooks")
        hook = [None]
        m.set_axon_ntff_profile_hook = lambda h: hook.__setitem__(0, h)
        m.get_axon_ntff_profile_hook = lambda: hook[0]
        sys.modules["antenv.axon_hooks"] = m
        antenv.axon_hooks = m
        from trn_agent_boot.trn_boot import _ntff_profile_via_ctypes
        m.set_axon_ntff_profile_hook(
            _ntff_profile_via_ctypes("/opt/axon/libaxon_pjrt.so"))
    except Exception:
        pass


def _build_nc():
    import concourse.mybir as mybir
    import concourse.tile as tile
    from concourse import bacc

    f32, f16 = mybir.dt.float32, mybir.dt.float16
    EXP = mybir.ActivationFunctionType.Exp
    SQRT = mybir.ActivationFunctionType.Sqrt

    nc = bacc.Bacc("TRN2", target_bir_lowering=False, debug=False,
                   num_devices=NCORES)

    d_xt = nc.dram_tensor("xt", (16, 128, SEQ), f16, kind="ExternalInput")
    d_wq = nc.dram_tensor("wq", (16, 128, 256), f16, kind="ExternalInput")
    d_wkv = nc.dram_tensor("wkv", (16, 128, 128), f16, kind="ExternalInput")
    d_wo = nc.dram_tensor("wo", (2, 128, DIM), f16, kind="ExternalInput")
    d_cosq = nc.dram_tensor("cosq", (128, SEQ), f16, kind="ExternalInput")
    d_sinq = nc.dram_tensor("sinq", (128, SEQ), f16, kind="ExternalInput")
    d_cosk = nc.dram_tensor("cosk", (64, SEQ), f16, kind="ExternalInput")
    d_sink = nc.dram_tensor("sink", (64, SEQ), f16, kind="ExternalInput")
    d_tri = nc.dram_tensor("tri", (128, 128), f16, kind="ExternalInput")
    d_idn = nc.dram_tensor("idn", (64, 64), f16, kind="ExternalInput")
    d_ob = nc.dram_tensor("ob", (128, 2), f16, kind="ExternalInput")
    d_o64 = nc.dram_tensor("o64", (64, 1), f16, kind="ExternalInput")
    d_y = nc.dram_tensor("y", (16, 128, DIM), f16, kind="ExternalOutput")

    with tile.TileContext(nc) as tc:
        from contextlib import ExitStack
        with ExitStack() as ctx:
            kconst = ctx.enter_context(tc.tile_pool(name="kconst", bufs=1))
            xpool = ctx.enter_context(tc.tile_pool(name="xp", bufs=1))
            work = ctx.enter_context(tc.tile_pool(name="work", bufs=1))
            vpool = ctx.enter_context(tc.tile_pool(name="vp", bufs=1))
            epool = ctx.enter_context(tc.tile_pool(name="ep", bufs=3))
            flex = ctx.enter_context(
                tc.tile_pool(name="flex", bufs=2, space="PSUM"))
            outps = ctx.enter_context(
                tc.tile_pool(name="outps", bufs=2, space="PSUM"))

            # ---- constant loads -------------------------------------------
            xts = []
            for i in range(16):
                t = xpool.tile([128, SEQ], f16, tag=f"xt{i}", name=f"xt{i}")
                nc.sync.dma_start(out=t, in_=d_xt[i])
                xts.append(t)
            wq_sb = kconst.tile([128, 16, 256], f16, tag="wq")
            nc.sync.dma_start(out=wq_sb, in_=d_wq.ap().rearrange("i p c -> p i c"))
            wkv_sb = kconst.tile([128, 16, 128], f16, tag="wkv")
            nc.sync.dma_start(out=wkv_sb, in_=d_wkv.ap().rearrange("i p c -> p i c"))
            wo_sb = kconst.tile([128, 2, DIM], f16, tag="wo")
            nc.sync.dma_start(out=wo_sb, in_=d_wo.ap().rearrange("g p c -> p g c"))
            cosq_sb = kconst.tile([128, SEQ], f16, tag="cosq")
            nc.sync.dma_start(out=cosq_sb, in_=d_cosq[:, :])
            sinq_sb = kconst.tile([128, SEQ], f16, tag="sinq")
            nc.sync.dma_start(out=sinq_sb, in_=d_sinq[:, :])
            cosk_sb = kconst.tile([64, SEQ], f16, tag="cosk")
            nc.sync.dma_start(out=cosk_sb, in_=d_cosk[:, :])
            sink_sb = kconst.tile([64, SEQ], f16, tag="sink")
            nc.sync.dma_start(out=sink_sb, in_=d_sink[:, :])
            tri_sb = kconst.tile([128, 128], f16, tag="tri")
            nc.sync.dma_start(out=tri_sb, in_=d_tri[:, :])
            idn_sb = kconst.tile([64, 64], f16, tag="idn")
            nc.sync.dma_start(out=idn_sb, in_=d_idn[:, :])
            ob_sb = kconst.tile([128, 2], f16, tag="ob")
            nc.sync.dma_start(out=ob_sb, in_=d_ob[:, :])
            o64_sb = kconst.tile([64, 1], f16, tag="o64")
            nc.sync.dma_start(out=o64_sb, in_=d_o64[:, :])
            eps_sb = kconst.tile([128, 1], f32, tag="eps")
            nc.vector.memset(eps_sb, EPS)
            eps64_sb = kconst.tile([128, 1], f32, tag="eps64")
            nc.vector.memset(eps64_sb, float(HD) * EPS)

            # persistent results of the projection phase
            qro = [kconst.tile([64, SEQ], f16, tag=f"qro{h}", name=f"qro{h}") for h in range(4)]
            kro = kconst.tile([64, SEQ], f16, tag="kro")
            rk2 = kconst.tile([128, 16], f32, tag="rk2")  # 0.125*rsqrt(ssq_k) cols
            vr = [vpool.tile([128, 65], f16, tag=f"vr{j}", name=f"vr{j}") for j in range(16)]
            aot = [kconst.tile([128, SEQ], f16, tag=f"aot{g}", name=f"aot{g}") for g in range(2)]

            # ---- projections + rope/norm ----------------------------------
            def proj_psum(lhsT_of_k):
                """Accumulate W.T @ xT into two (128, 1024) psum tiles."""
                pj = [flex.tile([128, CB], f32, tag="flex", name="pj") for _ in range(2)]
                for k in range(16):
                    w = lhsT_of_k(k)
                    for half in range(2):
                        for n2 in range(2):
                            c = CB * half + 512 * n2
                            nc.tensor.matmul(
                                pj[half][:, 512 * n2:512 * n2 + 512],
                                w, xts[k][:, c:c + 512],
                                start=(k == 0), stop=(k == 15))
                return pj

            def copy_out(pj, dst, rows=128):
                # split psum->sbuf copies between ACT and DVE
                nc.scalar.copy(out=dst[:rows, 0:CB], in_=pj[0][:rows, :])
                nc.vector.tensor_copy(out=dst[:rows, CB:SEQ], in_=pj[1][:rows, :])

            for g in range(2):  # q groups: heads 2g, 2g+1
                pj = proj_psum(lambda k, g=g: wq_sb[:, k, 128 * g:128 * g + 128])
                q_raw = work.tile([128, SEQ], f32, tag="qraw")
                copy_out(pj, q_raw)
                sq = work.tile([128, SEQ], f16, tag="sq")
                nc.vector.tensor_mul(sq, q_raw, q_raw)
                # per-head sum of squares -> sqrt -> reciprocal, rows kept base-0
                rs16 = [work.tile([1, SEQ], f16, tag=f"rs{r}", name=f"rs{r}") for r in range(2)]
                for n in range(4):
                    for r in range(2):
                        sp = outps.tile([1, 512], f32, tag="outps")
                        nc.tensor.matmul(sp, ob_sb[:, r:r + 1],
                                         sq[:, 512 * n:512 * n + 512],
                                         start=True, stop=True)
                        nc.scalar.activation(
                            out=rs16[r][:, 512 * n:512 * n + 512], in_=sp,
                            func=SQRT, scale=1.0 / HD, bias=eps_sb[0:1, :])
                # partition_broadcast writes garbage at out base!=0, and
                # 2-input SB ops need equal base partitions -> keep every
                # 2-input op and pb target at base 0 (per-head 64-row tiles).
                bqh = []
                for r in range(2):
                    with nc.allow_low_precision(reason="rms scale f16 ok"):
                        nc.vector.reciprocal(out=rs16[r], in_=rs16[r])
                    b = work.tile([64, SEQ], f16, tag=f"bqh{r}", name=f"bqh{r}")
                    nc.gpsimd.partition_broadcast(b, rs16[r], channels=64)
                    bqh.append(b)
                # rope: rot = shifted(q) (gpsimd copies are exempt from the
                # equal-base rule); per-head halves, all outputs base 0
                rot = work.tile([128, SEQ], f16, tag="rot")
                for (o, s) in ((0, 32), (32, 0), (64, 96), (96, 64)):
                    nc.gpsimd.tensor_copy(out=rot[o:o + 32, :],
                                          in_=q_raw[s:s + 32, :])
                for r in range(2):
                    tmph = work.tile([64, SEQ], f16, tag="tmph", bufs=2)
                    nc.vector.tensor_mul(tmph, rot[64 * r:64 * r + 64, :],
                                         sinq_sb[64 * r:64 * r + 64, :])
                    hhh = work.tile([64, SEQ], f16, tag="hhh", bufs=2)
                    nc.vector.tensor_mul(hhh, q_raw[64 * r:64 * r + 64, :],
                                         cosq_sb[64 * r:64 * r + 64, :])
                    nc.vector.tensor_add(hhh, hhh, tmph)
                    nc.vector.tensor_mul(qro[2 * g + r], hhh, bqh[r])

            # kv group
            pj = proj_psum(lambda k: wkv_sb[:, k, :])
            kv_raw = work.tile([128, SEQ], f32, tag="qraw")
            copy_out(pj, kv_raw)
            sqk = work.tile([64, SEQ], f16, tag="sq")
            nc.vector.tensor_mul(sqk, kv_raw[0:64, :], kv_raw[0:64, :])
            pc = outps.tile([128, 16], f32, tag="outps")
            for j in range(16):
                nc.tensor.matmul(pc[:, j:j + 1],
                                 sqk[:, 128 * j:128 * j + 128], o64_sb,
                                 start=True, stop=True)
            # 0.125*rsqrt(ssq/64 + eps) == rsqrt(ssq + 64*eps)
            nc.scalar.activation(out=rk2, in_=pc, func=SQRT,
                                 scale=1.0, bias=eps64_sb)
            nc.vector.reciprocal(out=rk2, in_=rk2)
            rotk = work.tile([64, SEQ], f16, tag="rot")
            for (o, s) in ((0, 32), (32, 0)):
                nc.gpsimd.tensor_copy(out=rotk[o:o + 32, :],
                                      in_=kv_raw[s:s + 32, :])
            tmpk = work.tile([64, SEQ], f16, tag="tmph", bufs=2)
            nc.vector.tensor_mul(tmpk, rotk[0:64, :], sink_sb)
            hk = work.tile([64, SEQ], f16, tag="hhh", bufs=2)
            nc.vector.tensor_mul(hk, kv_raw[0:64, :], cosk_sb)
            nc.vector.tensor_add(kro, hk, tmpk)
            # v: cast (shifted copy) then 16 PE transposes into [v|1] row tiles
            v16 = work.tile([64, SEQ], f16, tag="v16")
            nc.vector.tensor_copy(out=v16, in_=kv_raw[64:128, :])
            for j in range(16):
                tp = outps.tile([128, 64], f16, tag="outps")
                nc.tensor.transpose(tp, v16[:, 128 * j:128 * j + 128], idn_sb)
                nc.vector.tensor_copy(out=vr[j][:, 0:64], in_=tp)
                nc.vector.memset(vr[j][:, 64:65], 1.0)

            # ---- attention + output, per column block --------------------
            for cb in range(2):
                c0 = CB * cb
                for h in range(4):
                    ot = outps.tile([65, CB], f32, tag="outps")
                    jmax = 8 * cb + 7
                    for j in range(jmax + 1):
                        p0 = max(128 * j, c0)
                        sc = flex.tile([128, CB], f32, tag="flex")
                        q = p0
                        while q < c0 + CB:
                            qe = min(c0 + CB, (q // 512 + 1) * 512)
                            nc.tensor.matmul(sc[:, q - c0:qe - c0],
                                             kro[:, 128 * j:128 * j + 128],
                                             qro[h][:, q:qe],
                                             start=True, stop=True)
                            q = qe
                        ex = epool.tile([128, CB], f16, tag="ex")
                        nc.scalar.activation(out=ex[:, p0 - c0:CB],
                                             in_=sc[:, p0 - c0:CB],
                                             func=EXP, scale=rk2[:, j:j + 1])
                        if 128 * j >= c0:  # diagonal block: zero ks>qs
                            nc.vector.tensor_mul(ex[:, p0 - c0:p0 - c0 + 128],
                                                 ex[:, p0 - c0:p0 - c0 + 128],
                                                 tri_sb)
                        q = p0
                        while q < c0 + CB:
                            qe = min(c0 + CB, (q // 512 + 1) * 512)
                            nc.tensor.matmul(ot[:, q - c0:qe - c0],
                                             vr[j],
                                             ex[:, q - c0:qe - c0],
                                             start=(j == 0), stop=(j == jmax))
                            q = qe
                    rden = work.tile([1, CB], f16, tag="rden", bufs=2)
                    with nc.allow_low_precision(reason="softmax recip f16 ok"):
                        nc.vector.reciprocal(out=rden, in_=ot[64:65, :])
                    bs = work.tile([64, CB], f16, tag="bs", bufs=2)
                    nc.gpsimd.partition_broadcast(bs, rden, channels=64)
                    g, r = h // 2, h % 2
                    nc.vector.tensor_mul(
                        aot[g][64 * r:64 * r + 64, c0:c0 + CB],
                        ot[0:64, :], bs)
                # Wo for the 8 seq chunks of this column block
                for m in range(8 * cb, 8 * cb + 8):
                    for hd_ in range(2):  # output dim halves
                        yp = flex.tile([128, CB], f32, tag="flex")
                        for g in range(2):
                            for n2 in range(2):
                                c = CB * hd_ + 512 * n2
                                nc.tensor.matmul(
                                    yp[:, 512 * n2:512 * n2 + 512],
                                    aot[g][:, 128 * m:128 * m + 128],
                                    wo_sb[:, g, c:c + 512],
                                    start=(g == 0), stop=(g == 1))
                        ys = work.tile([128, CB], f16, tag="ys", bufs=3)
                        if hd_ == 0:
                            nc.vector.tensor_copy(out=ys, in_=yp)
                        else:
                            nc.scalar.copy(out=ys, in_=yp)
                        nc.sync.dma_start(
                            out=d_y[m][:, CB * hd_:CB * hd_ + CB], in_=ys)
    nc.compile()
    return nc


def _get_nc():
    if "nc" not in _CACHE:
        _ensure_ntff_hook()
        _CACHE["nc"] = _build_nc()
    return _CACHE["nc"]


def _make_tables(qn_w, kn_w, start_pos):
    inv = THETA ** (-np.arange(0, HD, 2, dtype=np.float64) / HD)  # (32,)
    pos = float(start_pos) + np.arange(SEQ, dtype=np.float64)
    ang = inv[:, None] * pos[None, :]  # (32, SEQ)
    c, s = np.cos(ang), np.sin(ang)

    def tabs(gain):
        g = gain.astype(np.float64)
        cosg = np.concatenate([g[0:32, None] * c, g[32:64, None] * c], axis=0)
        sing = np.concatenate([-g[32:64, None] * s, g[0:32, None] * s], axis=0)
        return cosg.astype(np.float16), sing.astype(np.float16)

    cq, sq_ = tabs(np.asarray(qn_w))
    ck, sk = tabs(np.asarray(kn_w))
    return (np.ascontiguousarray(np.tile(cq, (2, 1))),
            np.ascontiguousarray(np.tile(sq_, (2, 1))), ck, sk)


def _prep_in_maps(x, Wq, Wk, Wv, Wo, qn_w, kn_w, start_pos):
    xT = np.ascontiguousarray(np.asarray(x)[0].T).astype(np.float16)
    xt = xT.reshape(16, 128, SEQ)
    cosq, sinq, cosk, sink = _make_tables(qn_w, kn_w, start_pos)
    tri = np.triu(np.ones((128, 128), np.float16))
    idn = np.eye(64, dtype=np.float16)
    ob = np.zeros((128, 2), np.float16)
    ob[0:64, 0] = 1.0
    ob[64:128, 1] = 1.0
    o64 = np.ones((64, 1), np.float16)
    Wq, Wk, Wv, Wo = (np.asarray(a) for a in (Wq, Wk, Wv, Wo))
    in_maps = []
    for c in range(NCORES):
        wq_c = np.ascontiguousarray(
            Wq[:, 256 * c:256 * (c + 1)]).astype(np.float16).reshape(16, 128, 256)
        wkv_c = np.ascontiguousarray(np.concatenate(
            [Wk[:, HD * c:HD * (c + 1)], Wv[:, HD * c:HD * (c + 1)]],
            axis=1)).astype(np.float16).reshape(16, 128, 128)
        wo_c = np.ascontiguousarray(
            Wo[256 * c:256 * (c + 1), :]).astype(np.float16).reshape(2, 128, DIM)
        in_maps.append({"xt": xt, "wq": wq_c, "wkv": wkv_c, "wo": wo_c,
                        "cosq": cosq, "sinq": sinq, "cosk": cosk, "sink": sink,
                        "tri": tri, "idn": idn, "ob": ob, "o64": o64})
    return in_maps


def run(inputs, trace=False, **kw):
    from concourse import bass_utils
    nc = _get_nc()
    in_maps = _prep_in_maps(
        inputs["x"], inputs["Wq"], inputs["Wk"], inputs["Wv"], inputs["Wo"],
        inputs["qn_w"], inputs["kn_w"], inputs["start_pos"])
    res = bass_utils.run_bass_kernel_spmd(
        nc, in_maps, core_ids=list(range(NCORES)), trace=trace, **kw)
    y = np.zeros((SEQ, DIM), np.float32)
    for r in res.results:
        y += r["y"].reshape(SEQ, DIM).astype(np.float32)
    return y.reshape(1, SEQ, DIM), res


def kernel(x, Wq, Wk, Wv, Wo, qn_w, kn_w, mask, start_pos):
    out, _ = run(dict(x=x, Wq=Wq, Wk=Wk, Wv=Wv, Wo=Wo, qn_w=qn_w, kn_w=kn_w,
                      mask=mask, start_pos=start_pos))
    return out


# revision 11
# speedup vs baseline: 1.2334x; 1.2334x over previous
"""GQA kernel for trn2, 8 NeuronCores, tensor-parallel over heads.

Sharding: 4 q heads + 1 kv head per core (column-split Wq/Wk/Wv, row-split Wo),
partial outputs summed on host. All matmul operands fp16, fp32 PSUM accumulation.

Layout strategy (everything "T" = feature-on-partitions):
  qT/kT/vT projections:  out[f, s] = W[:, f].T @ xT[:, s]  (xT host-pretransposed)
  scoresT[ks, qs] = kroT.T @ qroT  per 128-wide ks chunk    (contraction over head_dim)
  exp via ACT with per-partition scale = 0.125 * rsqrt(ssq_k) (k RMS-norm folded in)
  outT[d, qs] accumulated as [v|1].T @ expT  -> row 64 is the softmax denominator
  y[s, :] = attn_outT.T @ Wo  per 128-row seq chunk
RoPE + QK-norm gains folded into host-precomputed cos/sin tables.
"""

import sys
import types
import numpy as np
import ml_dtypes

for _p in ("/opt/trn_rl_repo",):
    if _p not in sys.path:
        sys.path.append(_p)

SEQ = 2048
DIM = 2048
HD = 64
NCORES = 8
HPC = 4  # q heads per core
EPS = 1e-6
THETA = 10000.0
CB = 1024  # attention column block (2 PSUM banks wide)

_CACHE = {}


def _ensure_ntff_hook():
    """Re-register the NTFF profile hook the boot drops (stub antenv)."""
    if "antenv.axon_hooks" in sys.modules:
        return
    try:
        import antenv
        m = types.ModuleType("antenv.axon_hooks")
        hook = [None]
        m.set_axon_ntff_profile_hook = lambda h: hook.__setitem__(0, h)
        m.get_axon_ntff_profile_hook = lambda: hook[0]
        sys.modules["antenv.axon_hooks"] = m
        antenv.axon_hooks = m
        from trn_agent_boot.trn_boot import _ntff_profile_via_ctypes
        m.set_axon_ntff_profile_hook(
            _ntff_profile_via_ctypes("/opt/axon/libaxon_pjrt.so"))
    except Exception:
        pass


def _build_nc():
    import concourse.mybir as mybir
    import concourse.tile as tile
    from concourse import bacc

    f32, f16 = mybir.dt.float32, mybir.dt.bfloat16
    EXP = mybir.ActivationFunctionType.Exp
    RSQ = mybir.ActivationFunctionType.Abs_reciprocal_sqrt

    nc = bacc.Bacc("TRN2", target_bir_lowering=False, debug=False,
                   num_devices=NCORES)

    d_xt = nc.dram_tensor("xt", (16, 128, SEQ), f16, kind="ExternalInput")
    d_wq = nc.dram_tensor("wq", (16, 128, 256), f16, kind="ExternalInput")
    d_wkv = nc.dram_tensor("wkv", (16, 128, 128), f16, kind="ExternalInput")
    d_wo = nc.dram_tensor("wo", (2, 128, DIM), f16, kind="ExternalInput")
    d_cosq = nc.dram_tensor("cosq", (128, SEQ), f16, kind="ExternalInput")
    d_sinq = nc.dram_tensor("sinq", (128, SEQ), f16, kind="ExternalInput")
    d_cosk = nc.dram_tensor("cosk", (64, SEQ), f16, kind="ExternalInput")
    d_sink = nc.dram_tensor("sink", (64, SEQ), f16, kind="ExternalInput")
    d_tri = nc.dram_tensor("tri", (128, 128), f16, kind="ExternalInput")
    d_idn = nc.dram_tensor("idn", (64, 64), f16, kind="ExternalInput")
    d_ob = nc.dram_tensor("ob", (128, 2), f16, kind="ExternalInput")
    d_o64 = nc.dram_tensor("o64", (64, 1), f16, kind="ExternalInput")
    d_y = nc.dram_tensor("y", (16, 128, DIM), f16, kind="ExternalOutput")

    with tile.TileContext(nc) as tc:
        from contextlib import ExitStack
        with ExitStack() as ctx:
            kconst = ctx.enter_context(tc.tile_pool(name="kconst", bufs=1))
            xpool = ctx.enter_context(tc.tile_pool(name="xp", bufs=1))
            work = ctx.enter_context(tc.tile_pool(name="work", bufs=1))
            vpool = ctx.enter_context(tc.tile_pool(name="vp", bufs=1))
            epool = ctx.enter_context(tc.tile_pool(name="ep", bufs=2))
            flex = ctx.enter_context(
                tc.tile_pool(name="flex", bufs=2, space="PSUM"))
            outps = ctx.enter_context(
                tc.tile_pool(name="outps", bufs=2, space="PSUM"))

            # ---- constant loads -------------------------------------------
            xts = []
            for i in range(16):
                t = xpool.tile([128, SEQ], f16, tag=f"xt{i}", name=f"xt{i}")
                nc.sync.dma_start(out=t, in_=d_xt[i])
                xts.append(t)
            wq_sb = kconst.tile([128, 16, 256], f16, tag="wq")
            nc.sync.dma_start(out=wq_sb, in_=d_wq.ap().rearrange("i p c -> p i c"))
            wkv_sb = kconst.tile([128, 16, 128], f16, tag="wkv")
            nc.sync.dma_start(out=wkv_sb, in_=d_wkv.ap().rearrange("i p c -> p i c"))
            wo_sb = kconst.tile([128, 2, DIM], f16, tag="wo")
            nc.sync.dma_start(out=wo_sb, in_=d_wo.ap().rearrange("g p c -> p g c"))
            cosq_sb = kconst.tile([128, SEQ], f16, tag="cosq")
            nc.sync.dma_start(out=cosq_sb, in_=d_cosq[:, :])
            sinq_sb = kconst.tile([128, SEQ], f16, tag="sinq")
            nc.sync.dma_start(out=sinq_sb, in_=d_sinq[:, :])
            cosk_sb = kconst.tile([64, SEQ], f16, tag="cosk")
            nc.sync.dma_start(out=cosk_sb, in_=d_cosk[:, :])
            sink_sb = kconst.tile([64, SEQ], f16, tag="sink")
            nc.sync.dma_start(out=sink_sb, in_=d_sink[:, :])
            tri_sb = kconst.tile([128, 128], f16, tag="tri")
            nc.sync.dma_start(out=tri_sb, in_=d_tri[:, :])
            idn_sb = kconst.tile([64, 64], f16, tag="idn")
            nc.sync.dma_start(out=idn_sb, in_=d_idn[:, :])
            ob_sb = kconst.tile([128, 2], f16, tag="ob")
            nc.sync.dma_start(out=ob_sb, in_=d_ob[:, :])
            o64_sb = kconst.tile([64, 1], f16, tag="o64")
            nc.sync.dma_start(out=o64_sb, in_=d_o64[:, :])
            eps_sb = kconst.tile([128, 1], f32, tag="eps")
            nc.vector.memset(eps_sb, EPS)
            eps64_sb = kconst.tile([128, 1], f32, tag="eps64")
            nc.vector.memset(eps64_sb, float(HD) * EPS)

            # persistent results of the projection phase
            qro = [kconst.tile([64, SEQ], f16, tag=f"qro{h}", name=f"qro{h}") for h in range(4)]
            kro = kconst.tile([64, SEQ], f16, tag="kro")
            rk2 = kconst.tile([128, 16], f32, tag="rk2")  # 0.125*rsqrt(ssq_k) cols
            vr = [vpool.tile([128, 65], f16, tag=f"vr{j}", name=f"vr{j}") for j in range(16)]
            aot = [kconst.tile([128, SEQ], f16, tag=f"aot{g}", name=f"aot{g}") for g in range(2)]

            # ---- projections + rope/norm ----------------------------------
            def proj_psum(lhsT_of_k):
                """Accumulate W.T @ xT into two (128, 1024) psum tiles."""
                pj = [flex.tile([128, CB], f32, tag="flex", name="pj") for _ in range(2)]
                for k in range(16):
                    w = lhsT_of_k(k)
                    for half in range(2):
                        for n2 in range(2):
                            c = CB * half + 512 * n2
                            nc.tensor.matmul(
                                pj[half][:, 512 * n2:512 * n2 + 512],
                                w, xts[k][:, c:c + 512],
                                start=(k == 0), stop=(k == 15))
                return pj

            def copy_out(pj, dst, rows=128):
                # split psum->sbuf copies between ACT and DVE
                nc.scalar.copy(out=dst[:rows, 0:CB], in_=pj[0][:rows, :])
                nc.vector.tensor_copy(out=dst[:rows, CB:SEQ], in_=pj[1][:rows, :])

            for g in range(2):  # q groups: heads 2g, 2g+1
                pj = proj_psum(lambda k, g=g: wq_sb[:, k, 128 * g:128 * g + 128])
                q_raw = work.tile([128, SEQ], f32, tag="qraw")
                copy_out(pj, q_raw)
                sq = work.tile([128, SEQ], f16, tag="sq")
                nc.vector.tensor_mul(sq, q_raw, q_raw)
                # per-head sum of squares -> sqrt -> reciprocal, rows kept base-0
                rs16 = [work.tile([1, SEQ], f16, tag=f"rs{r}", name=f"rs{r}") for r in range(2)]
                for n in range(4):
                    for r in range(2):
                        sp = outps.tile([1, 512], f32, tag="outps")
                        nc.tensor.matmul(sp, ob_sb[:, r:r + 1],
                                         sq[:, 512 * n:512 * n + 512],
                                         start=True, stop=True)
                        nc.scalar.activation(
                            out=rs16[r][:, 512 * n:512 * n + 512], in_=sp,
                            func=RSQ, scale=1.0 / HD, bias=eps_sb[0:1, :])
                # partition_broadcast writes garbage at out base!=0, and
                # 2-input SB ops need equal base partitions -> keep every
                # 2-input op and pb target at base 0 (per-head 64-row tiles).
                bqh = []
                for r in range(2):
                    b = work.tile([64, SEQ], f16, tag=f"bqh{r}", name=f"bqh{r}")
                    nc.gpsimd.partition_broadcast(b, rs16[r], channels=64)
                    bqh.append(b)
                # rope: rot = shifted(q) (gpsimd copies are exempt from the
                # equal-base rule); per-head halves, all outputs base 0
                q16 = work.tile([128, SEQ], f16, tag="q16")
                nc.scalar.copy(out=q16, in_=q_raw)
                rot = work.tile([128, SEQ], f16, tag="rot")
                for (o, s) in ((0, 32), (32, 0), (64, 96), (96, 64)):
                    nc.vector.tensor_copy(out=rot[o:o + 32, :],
                                          in_=q16[s:s + 32, :])
                for r in range(2):
                    tmph = work.tile([64, SEQ], f16, tag="tmph", bufs=2)
                    nc.vector.tensor_mul(tmph, rot[64 * r:64 * r + 64, :],
                                         sinq_sb[64 * r:64 * r + 64, :])
                    hhh = work.tile([64, SEQ], f16, tag="hhh", bufs=2)
                    nc.vector.tensor_mul(hhh, q16[64 * r:64 * r + 64, :],
                                         cosq_sb[64 * r:64 * r + 64, :])
                    nc.vector.tensor_add(hhh, hhh, tmph)
                    nc.vector.tensor_mul(qro[2 * g + r], hhh, bqh[r])

            # kv group
            pj = proj_psum(lambda k: wkv_sb[:, k, :])
            kv_raw = work.tile([128, SEQ], f32, tag="qraw")
            copy_out(pj, kv_raw)
            sqk = work.tile([64, SEQ], f16, tag="sq")
            nc.vector.tensor_mul(sqk, kv_raw[0:64, :], kv_raw[0:64, :])
            pc = outps.tile([128, 16], f32, tag="outps")
            for j in range(16):
                nc.tensor.matmul(pc[:, j:j + 1],
                                 sqk[:, 128 * j:128 * j + 128], o64_sb,
                                 start=True, stop=True)
            # 0.125*rsqrt(ssq/64 + eps) == rsqrt(ssq + 64*eps)
            nc.scalar.activation(out=rk2, in_=pc, func=RSQ,
                                 scale=1.0, bias=eps64_sb)
            k16 = work.tile([64, SEQ], f16, tag="q16")
            nc.scalar.copy(out=k16, in_=kv_raw[0:64, :])
            rotk = work.tile([64, SEQ], f16, tag="rot")
            for (o, s) in ((0, 32), (32, 0)):
                nc.vector.tensor_copy(out=rotk[o:o + 32, :],
                                      in_=k16[s:s + 32, :])
            tmpk = work.tile([64, SEQ], f16, tag="tmph", bufs=2)
            nc.vector.tensor_mul(tmpk, rotk[0:64, :], sink_sb)
            hk = work.tile([64, SEQ], f16, tag="hhh", bufs=2)
            nc.vector.tensor_mul(hk, k16, cosk_sb)
            nc.vector.tensor_add(kro, hk, tmpk)
            # v: cast (shifted copy) then 16 PE transposes into [v|1] row tiles
            v16 = work.tile([64, SEQ], f16, tag="v16")
            nc.vector.tensor_copy(out=v16, in_=kv_raw[64:128, :])
            for j in range(16):
                tp = outps.tile([128, 64], f16, tag="outps")
                nc.tensor.transpose(tp, v16[:, 128 * j:128 * j + 128], idn_sb)
                nc.vector.tensor_copy(out=vr[j][:, 0:64], in_=tp)
                nc.vector.memset(vr[j][:, 64:65], 1.0)

            # ---- attention + output, per column block --------------------
            for cb in range(2):
                c0 = CB * cb
                for h in range(4):
                    ot = outps.tile([65, CB], f32, tag="outps")
                    jmax = 8 * cb + 7
                    for j in range(jmax + 1):
                        p0 = max(128 * j, c0)
                        sc = flex.tile([128, CB], f32, tag="flex")
                        q = p0
                        while q < c0 + CB:
                            qe = min(c0 + CB, (q // 512 + 1) * 512)
                            nc.tensor.matmul(sc[:, q - c0:qe - c0],
                                             kro[:, 128 * j:128 * j + 128],
                                             qro[h][:, q:qe],
                                             start=True, stop=True)
                            q = qe
                        ex = epool.tile([128, CB], f16, tag="ex")
                        nc.scalar.activation(out=ex[:, p0 - c0:CB],
                                             in_=sc[:, p0 - c0:CB],
                                             func=EXP, scale=rk2[:, j:j + 1])
                        if 128 * j >= c0:  # diagonal block: zero ks>qs
                            nc.vector.tensor_mul(ex[:, p0 - c0:p0 - c0 + 128],
                                                 ex[:, p0 - c0:p0 - c0 + 128],
                                                 tri_sb)
                        q = p0
                        while q < c0 + CB:
                            qe = min(c0 + CB, (q // 512 + 1) * 512)
                            nc.tensor.matmul(ot[:, q - c0:qe - c0],
                                             vr[j],
                                             ex[:, q - c0:qe - c0],
                                             start=(j == 0), stop=(j == jmax))
                            q = qe
                    dsq = work.tile([1, CB], f32, tag="dsq", bufs=1)
                    nc.scalar.activation(out=dsq, in_=ot[64:65, :],
                                         func=mybir.ActivationFunctionType.Square,
                                         scale=1.0, bias=0.0)
                    rden = work.tile([1, CB], f16, tag="rden", bufs=1)
                    nc.scalar.activation(out=rden, in_=dsq, func=RSQ,
                                         scale=1.0, bias=0.0)
                    bs = work.tile([64, CB], f16, tag="bs", bufs=2)
                    nc.gpsimd.partition_broadcast(bs, rden, channels=64)
                    g, r = h // 2, h % 2
                    nc.vector.tensor_mul(
                        aot[g][64 * r:64 * r + 64, c0:c0 + CB],
                        ot[0:64, :], bs)
                # Wo for the 8 seq chunks of this column block
                for m in range(8 * cb, 8 * cb + 8):
                    for hd_ in range(2):  # output dim halves
                        yp = flex.tile([128, CB], f32, tag="flex")
                        for g in range(2):
                            for n2 in range(2):
                                c = CB * hd_ + 512 * n2
                                nc.tensor.matmul(
                                    yp[:, 512 * n2:512 * n2 + 512],
                                    aot[g][:, 128 * m:128 * m + 128],
                                    wo_sb[:, g, c:c + 512],
                                    start=(g == 0), stop=(g == 1))
                        ys = work.tile([128, CB], f16, tag="ys", bufs=2)
                        if hd_ == 0:
                            nc.vector.tensor_copy(out=ys, in_=yp)
                        else:
                            nc.scalar.copy(out=ys, in_=yp)
                        nc.sync.dma_start(
                            out=d_y[m][:, CB * hd_:CB * hd_ + CB], in_=ys)
    nc.compile()
    return nc


def _get_nc():
    if "nc" not in _CACHE:
        _ensure_ntff_hook()
        _CACHE["nc"] = _build_nc()
    return _CACHE["nc"]


def _make_tables(qn_w, kn_w, start_pos):
    inv = THETA ** (-np.arange(0, HD, 2, dtype=np.float64) / HD)  # (32,)
    pos = float(start_pos) + np.arange(SEQ, dtype=np.float64)
    ang = inv[:, None] * pos[None, :]  # (32, SEQ)
    c, s = np.cos(ang), np.sin(ang)

    def tabs(gain):
        g = gain.astype(np.float64)
        cosg = np.concatenate([g[0:32, None] * c, g[32:64, None] * c], axis=0)
        sing = np.concatenate([-g[32:64, None] * s, g[0:32, None] * s], axis=0)
        return cosg.astype(ml_dtypes.bfloat16), sing.astype(ml_dtypes.bfloat16)

    cq, sq_ = tabs(np.asarray(qn_w))
    ck, sk = tabs(np.asarray(kn_w))
    return (np.ascontiguousarray(np.tile(cq, (2, 1))),
            np.ascontiguousarray(np.tile(sq_, (2, 1))), ck, sk)


def _prep_in_maps(x, Wq, Wk, Wv, Wo, qn_w, kn_w, start_pos):
    xT = np.ascontiguousarray(np.asarray(x)[0].T).astype(ml_dtypes.bfloat16)
    xt = xT.reshape(16, 128, SEQ)
    cosq, sinq, cosk, sink = _make_tables(qn_w, kn_w, start_pos)
    tri = np.triu(np.ones((128, 128), ml_dtypes.bfloat16))
    idn = np.eye(64, dtype=ml_dtypes.bfloat16)
    ob = np.zeros((128, 2), ml_dtypes.bfloat16)
    ob[0:64, 0] = 1.0
    ob[64:128, 1] = 1.0
    o64 = np.ones((64, 1), ml_dtypes.bfloat16)
    Wq, Wk, Wv, Wo = (np.asarray(a) for a in (Wq, Wk, Wv, Wo))
    in_maps = []
    for c in range(NCORES):
        wq_c = np.ascontiguousarray(
            Wq[:, 256 * c:256 * (c + 1)]).astype(ml_dtypes.bfloat16).reshape(16, 128, 256)
        wkv_c = np.ascontiguousarray(np.concatenate(
            [Wk[:, HD * c:HD * (c + 1)], Wv[:, HD * c:HD * (c + 1)]],
            axis=1)).astype(ml_dtypes.bfloat16).reshape(16, 128, 128)
        wo_c = np.ascontiguousarray(
            Wo[256 * c:256 * (c + 1), :]).astype(ml_dtypes.bfloat16).reshape(2, 128, DIM)
        in_maps.append({"xt": xt, "wq": wq_c, "wkv": wkv_c, "wo": wo_c,
                        "cosq": cosq, "sinq": sinq, "cosk": cosk, "sink": sink,
                        "tri": tri, "idn": idn, "ob": ob, "o64": o64})
    return in_maps


def run(inputs, trace=False, **kw):
    from concourse import bass_utils
    nc = _get_nc()
    in_maps = _prep_in_maps(
        inputs["x"], inputs["Wq"], inputs["Wk"], inputs["Wv"], inputs["Wo"],
        inputs["qn_w"], inputs["kn_w"], inputs["start_pos"])
    res = bass_utils.run_bass_kernel_spmd(
        nc, in_maps, core_ids=list(range(NCORES)), trace=trace, **kw)
    y = np.zeros((SEQ, DIM), np.float32)
    for r in res.results:
        y += r["y"].reshape(SEQ, DIM).astype(np.float32)
    return y.reshape(1, SEQ, DIM), res


def kernel(x, Wq, Wk, Wv, Wo, qn_w, kn_w, mask, start_pos):
    out, _ = run(dict(x=x, Wq=Wq, Wk=Wk, Wv=Wv, Wo=Wo, qn_w=qn_w, kn_w=kn_w,
                      mask=mask, start_pos=start_pos))
    return out
